# revision 8
# baseline (speedup 1.0000x reference)
"""Trainium2 Bass kernel for nn_CapGATattentionGRU (8-core SPMD).

Math notes exploited here:
- The reference GRU scans a length-1 sequence with h0 = 0, so the
  (3F x F) W_hh matmuls reduce to their biases b_hh.  Only W_ih0/W_ih1
  (100 MB total) need to be streamed.
- Tensor-parallel sharding: each core owns 256 output features per gate
  (columns of gi) for both GRU layers; hidden states are AllGathered
  between layers.  Everything after the GRU (attention over T=12, GAT on
  128 nodes, capsule routing) is tiny and runs replicated on all cores.
"""

import os
import numpy as np

I, H, T, F = 128, 16, 12, 2048
NCORES = 8
GPC = F // NCORES          # 256 gate-features per core
KT = F // 128              # 16 k-tiles of the contraction dim
NCHUNK = 4                 # weight DMA chunks per layer

_STATE = {}


# --------------------------------------------------------------------------
# device module
# --------------------------------------------------------------------------
def _build_module():
    from contextlib import ExitStack

    import concourse.bass as bass
    import concourse.tile as tile
    from concourse import bacc, mybir
    from concourse.masks import make_identity

    dt = mybir.dt.float32
    X = mybir.AxisListType.X
    AF = mybir.ActivationFunctionType
    OP = mybir.AluOpType
    AP = bass.AP

    nc = bacc.Bacc(
        "TRN2",
        target_bir_lowering=False,
        debug=False,
        num_devices=NCORES,
    )

    def din(name, shape):
        return nc.dram_tensor(name, list(shape), dt, kind="ExternalInput").ap()

    w_dram = [
        [din(f"w{layer}_{j}", (128, 12 * KT * 16 * 4 // NCHUNK)) for j in range(NCHUNK)]
        for layer in range(2)
    ]  # each (128, 3072): k-major [k, j] with j in 0..768
    xT_d = din("xT", (128, KT * T))
    bias_d = [din(f"bias{layer}", (128, 8)) for layer in range(2)]
    wattT_d = din("wattT", (T, T))
    batt_d = din("batt", (T, 1))
    gat_d = []
    for layer in range(2):
        gat_d.append(
            dict(
                wlT=din(f"wlT{layer}", (H, H)),
                wrT=din(f"wrT{layer}", (H, H)),
                bl=din(f"bl{layer}", (H, 1)),
                br=din(f"br{layer}", (H, 1)),
                gb=din(f"gb{layer}", (H, 1)),
                abc=din(f"abc{layer}", (128, H)),
            )
        )
    wc1T_d = din("wc1T", (H, 2048))
    wc2T_d = din("wc2T", (H, 2048))
    wfusT_d = din("wfusT", (H, 128))
    bfus_d = din("bfus", (128, 128))
    out_d = nc.dram_tensor("out", [128, 128], dt, kind="ExternalOutput").ap()

    with ExitStack() as ctx:
        tc = ctx.enter_context(tile.TileContext(nc))
        const = ctx.enter_context(tc.tile_pool(name="const", bufs=1))
        work = ctx.enter_context(tc.tile_pool(name="work", bufs=1))
        psum = ctx.enter_context(tc.tile_pool(name="psum", bufs=1, space="PSUM"))
        dram = ctx.enter_context(tc.tile_pool(name="dram", bufs=1, space="DRAM"))

        def wtile(shape, tag):
            return work.tile(list(shape), dt, tag=tag, name=tag)

        def ptile(shape, i):
            return psum.tile(list(shape), dt, tag=f"P{i}", name=f"P{i}")

        def ttile(shape):
            return psum.tile(list(shape), dt, tag="tr", name="tr", bufs=2)

        def bc_free(ap, dims):
            """Manual AP: keep partition dim, replace free dims with [step,count]s."""
            return AP(tensor=ap.tensor, offset=ap.offset, ap=[ap.ap[0]] + list(dims))

        # ---- big weight DMAs first (HWDGE, streams in order) -------------
        w_sb = []
        for layer in range(2):
            tiles = []
            for j in range(NCHUNK):
                t = const.tile([128, 3072], dt, tag=f"w{layer}_{j}", name=f"w{layer}_{j}")
                nc.sync.dma_start(out=t[:], in_=w_dram[layer][j])
                tiles.append(t)
            w_sb.append(tiles)

        # ---- small constants (SWDGE queues, overlap the weight stream) ---
        def load(ap_dram, tag):
            t = const.tile(list(ap_dram.shape), dt, tag=tag, name=tag)
            nc.gpsimd.dma_start(out=t[:], in_=ap_dram)
            return t

        xT_sb = load(xT_d, "xT")
        bias_sb = [load(bias_d[0], "bias0"), load(bias_d[1], "bias1")]
        wattT_sb = load(wattT_d, "wattT")
        batt_sb = load(batt_d, "batt")
        gat_sb = []
        for layer in range(2):
            gat_sb.append({k: load(v, f"gat{layer}_{k}") for k, v in gat_d[layer].items()})
        wc1T_sb = load(wc1T_d, "wc1T")
        wc2T_sb = load(wc2T_d, "wc2T")
        wfusT_sb = load(wfusT_d, "wfusT")
        bfus_sb = load(bfus_d, "bfus")

        ident = const.tile([128, 128], dt, tag="ident", name="ident")
        make_identity(nc, ident[:])
        ones1 = const.tile([1, 128], dt, tag="ones1", name="ones1")
        nc.vector.memset(ones1[:], 1.0)
        eps_t = const.tile([128, 1], dt, tag="eps_t", name="eps_t")
        nc.vector.memset(eps_t[:], 1e-8)

        # ---- GRU layers --------------------------------------------------
        h1T_sb = wtile((128, KT * T), "h1T")
        embT_sb = wtile((128, KT * T), "embT")
        d_slice = [dram.tile([2 * 128, T], dt, tag=f"dsl{layer}", name=f"dsl{layer}") for layer in range(2)]
        d_full = [dram.tile([F, T], dt, tag=f"dfull{layer}", name=f"dfull{layer}") for layer in range(2)]

        for layer in range(2):
            rhs3 = (xT_sb if layer == 0 else h1T_sb)[:].rearrange(
                "p (k t) -> p k t", k=KT
            )
            ps = [ptile([128, T], g * 2 + b) for g in range(3) for b in range(2)]
            for k in range(KT):
                ch, kk = k // 4, k % 4
                wv = w_sb[layer][ch][:].rearrange("p (k2 j) -> p k2 j", k2=4)
                for g in range(3):
                    for b in range(2):
                        nc.tensor.matmul(
                            ps[g * 2 + b][:],
                            lhsT=wv[:, kk, g * GPC + b * 128 : g * GPC + (b + 1) * 128],
                            rhs=rhs3[:, k, :],
                            start=(k == 0),
                            stop=(k == KT - 1),
                        )
            bl_sb = bias_sb[layer]
            for b in range(2):
                r_t = wtile((128, T), f"r{b}")
                nc.scalar.activation(r_t[:], ps[0 * 2 + b][:], AF.Sigmoid,
                                     bias=bl_sb[:, 0 + b : 1 + b])
                zc_t = wtile((128, T), f"zc{b}")
                nc.scalar.activation(zc_t[:], ps[1 * 2 + b][:], AF.Sigmoid,
                                     bias=bl_sb[:, 2 + b : 3 + b], scale=-1.0)
                t_t = wtile((128, T), f"t{b}")
                nc.vector.scalar_tensor_tensor(
                    out=t_t[:], in0=r_t[:], scalar=bl_sb[:, 6 + b : 7 + b],
                    in1=ps[2 * 2 + b][:], op0=OP.mult, op1=OP.add,
                )
                n_t = wtile((128, T), f"n{b}")
                nc.scalar.activation(n_t[:], t_t[:], AF.Tanh,
                                     bias=bl_sb[:, 4 + b : 5 + b])
                h_t = wtile((128, T), f"h{b}")
                nc.vector.tensor_mul(h_t[:], zc_t[:], n_t[:])
                if layer == 1:  # emb = relu(h2)
                    hr_t = wtile((128, T), f"hr{b}")
                    nc.scalar.activation(hr_t[:], h_t[:], AF.Relu)
                    h_t = hr_t
                nc.sync.dma_start(
                    out=d_slice[layer][b * 128 : (b + 1) * 128, :], in_=h_t[:]
                )
            nc.gpsimd.collective_compute(
                "AllGather",
                OP.bypass,
                replica_groups=[list(range(NCORES))],
                ins=[d_slice[layer][:].opt()],
                outs=[d_full[layer][:].opt()],
            )
            dst = h1T_sb if layer == 0 else embT_sb
            nc.sync.dma_start(
                out=dst[:].rearrange("p (k t) -> p k t", k=KT),
                in_=d_full[layer][:].rearrange("(k p) t -> p k t", p=128),
            )

        # ---- attention over T --------------------------------------------
        embT3 = embT_sb[:].rearrange("p (k t) -> p k t", k=KT)
        emb_nat = wtile((T, F), "emb_nat")
        for k in range(KT):
            tr = ttile([T, 128])
            nc.tensor.transpose(tr[:], embT3[:, k, :], ident[:])
            nc.scalar.copy(emb_nat[:, k * 128 : (k + 1) * 128], tr[:])

        awT_sb = wtile((T, F), "awT")
        for j in range(4):
            pa = ptile([T, 512], j)
            nc.tensor.matmul(pa[:], lhsT=wattT_sb[:],
                             rhs=emb_nat[:, j * 512 : (j + 1) * 512],
                             start=True, stop=True)
            nc.scalar.activation(awT_sb[:, j * 512 : (j + 1) * 512], pa[:],
                                 AF.Identity, bias=batt_sb[:])
        aw_sb = wtile((128, KT * T), "aw")
        aw3 = aw_sb[:].rearrange("p (k t) -> p k t", k=KT)
        for k in range(KT):
            tr = ttile([128, T])
            nc.tensor.transpose(tr[:], awT_sb[:, k * 128 : (k + 1) * 128],
                                ident[0:T, 0:T])
            nc.scalar.copy(aw3[:, k, :], tr[:])

        mx = wtile((128, KT), "mx")
        nc.vector.reduce_max(out=mx[:], in_=aw3, axis=X)
        exs = wtile((128, KT * T), "exs")
        ex3 = exs[:].rearrange("p (k t) -> p k t", k=KT)
        nc.vector.tensor_tensor(ex3, aw3, bc_free(mx[:], [[1, KT], [0, T]]), OP.subtract)
        nc.scalar.activation(exs[:], exs[:], AF.Exp)
        sm = wtile((128, KT), "sm")
        nc.vector.reduce_sum(out=sm[:], in_=ex3, axis=X)
        rs = wtile((128, KT), "rs")
        nc.vector.reciprocal(rs[:], sm[:])
        pe = wtile((128, KT * T), "pe")
        nc.vector.tensor_mul(pe[:], exs[:], embT_sb[:])
        num = wtile((128, KT), "num")
        nc.vector.reduce_sum(out=num[:], in_=pe[:].rearrange("p (k t) -> p k t", k=KT),
                             axis=X)
        attn = wtile((128, KT), "attn")
        nc.vector.tensor_mul(attn[:], num[:], rs[:])
        att_sb = wtile((128, KT), "att_sb")
        nc.scalar.activation(att_sb[:], attn[:], AF.Tanh)

        # (p, k) layout -> att_vec (i, h) via DRAM roundtrip
        trv = ttile([KT, 128])
        nc.tensor.transpose(trv[:], att_sb[:], ident[:])
        attT_sb = wtile((KT, 128), "attT")
        nc.scalar.copy(attT_sb[:], trv[:])
        d_att = dram.tile([KT, 128], dt, tag="datt", name="datt")
        nc.sync.dma_start(out=d_att[:], in_=attT_sb[:])
        att_vec = wtile((128, H), "att_vec")
        nc.sync.dma_start(
            out=att_vec[:],
            in_=d_att[:].rearrange("a b -> (a b)").rearrange("(i h) -> i h", h=H),
        )
        trv2 = ttile([H, 128])
        nc.tensor.transpose(trv2[:], att_vec[:], ident[:])
        attvT_sb = wtile((H, 128), "attvT")
        nc.scalar.copy(attvT_sb[:], trv2[:])

        # ---- GATv2 x2 ----------------------------------------------------
        def gat_layer(XT_tile, prm, tag):
            p_xl = ptile([H, 128], 0)
            nc.tensor.matmul(p_xl[:], lhsT=prm["wlT"][:], rhs=XT_tile[:],
                             start=True, stop=True)
            xlT = wtile((H, 128), f"xlT{tag}")
            nc.scalar.activation(xlT[:], p_xl[:], AF.Identity, bias=prm["bl"][:])
            p_xr = ptile([H, 128], 1)
            nc.tensor.matmul(p_xr[:], lhsT=prm["wrT"][:], rhs=XT_tile[:],
                             start=True, stop=True)
            xrT = wtile((H, 128), f"xrT{tag}")
            nc.scalar.activation(xrT[:], p_xr[:], AF.Identity, bias=prm["br"][:])

            p_t = ttile([128, H])
            nc.tensor.transpose(p_t[:], xlT[:], ident[0:H, 0:H])
            xl_nat = wtile((128, H), f"xln{tag}")
            nc.scalar.copy(xl_nat[:], p_t[:])

            xlf = wtile((1, 128 * H), "xlf")
            xlf_ap = xlf[:]
            nc.sync.dma_start(
                out=AP(tensor=xlf_ap.tensor, offset=xlf_ap.offset,
                       ap=[xlf_ap.ap[0], [H, 128], [1, H]]),
                in_=xl_nat[:],
            )
            e_sb = wtile((128, 128 * H), "e_sb")
            id_ap = ident[0:H, 0:H]
            id_rep = AP(tensor=id_ap.tensor, offset=id_ap.offset,
                        ap=[id_ap.ap[0], [0, 512 // H], id_ap.ap[1]])
            for j in range(4):
                p_e = ptile([128, 512], 2 + j)
                nc.tensor.matmul(p_e[:], lhsT=ones1[:],
                                 rhs=xlf[0:1, j * 512 : (j + 1) * 512],
                                 start=True, stop=False)
                nc.tensor.matmul(p_e[:], lhsT=xrT[:], rhs=id_rep,
                                 start=False, stop=True)
                # lrelu(v, 0.2) = 0.6*(v + (2/3)*|v|); the 0.6 is folded into
                # the host-side scaling of `a` (abc input carries 0.6*a).
                ab_t = wtile((128, 512), f"ab{j}")
                nc.scalar.activation(ab_t[:], p_e[:], AF.Abs)
                nc.vector.scalar_tensor_tensor(
                    out=e_sb[:, j * 512 : (j + 1) * 512], in0=ab_t[:],
                    scalar=2.0 / 3.0, in1=p_e[:], op0=OP.mult, op1=OP.add,
                )
            ew = wtile((128, 128 * H), "ew")
            abc_ap = prm["abc"][:]
            nc.vector.tensor_mul(
                ew[:].rearrange("p (s h) -> p s h", h=H),
                e_sb[:].rearrange("p (s h) -> p s h", h=H),
                bc_free(abc_ap, [[0, 128], [1, H]]),
            )
            spre = wtile((128, 128), "spre")
            nc.vector.reduce_sum(out=spre[:],
                                 in_=ew[:].rearrange("p (s h) -> p s h", h=H), axis=X)
            mx2 = wtile((128, 1), "mx2")
            nc.vector.reduce_max(out=mx2[:], in_=spre[:], axis=X, negate=True)
            ex2 = wtile((128, 128), "ex2")
            nc.scalar.activation(ex2[:], spre[:], AF.Exp, bias=mx2[:])
            sm2 = wtile((128, 1), "sm2")
            nc.vector.reduce_sum(out=sm2[:], in_=ex2[:], axis=X)
            rs2 = wtile((128, 1), "rs2")
            nc.vector.reciprocal(rs2[:], sm2[:])
            alph = wtile((128, 128), "alph")
            nc.vector.tensor_scalar_mul(alph[:], ex2[:], rs2[:])
            p_at = ttile([128, 128])
            nc.tensor.transpose(p_at[:], alph[:], ident[:])
            alphT = wtile((128, 128), "alphT")
            nc.scalar.copy(alphT[:], p_at[:])
            p_g = ptile([H, 128], 0)
            nc.tensor.matmul(p_g[:], lhsT=xl_nat[:], rhs=alphT[:],
                             start=True, stop=True)
            gT = wtile((H, 128), f"gT{tag}")
            nc.scalar.activation(gT[:], p_g[:], AF.Relu, bias=prm["gb"][:])
            return gT

        g0T = gat_layer(attvT_sb, gat_sb[0], "0")
        g1T = gat_layer(g0T, gat_sb[1], "1")
        gsumT = wtile((H, 128), "gsumT")
        nc.vector.tensor_add(gsumT[:], g0T[:], g1T[:])

        # ---- capsule priors + routing ------------------------------------
        P1 = wtile((128, H * 128), "P1")  # [o, (l, c)]
        for l in range(H):
            pc = ptile([128, 128], l % 2)
            nc.tensor.matmul(pc[:], lhsT=wc1T_sb[:, l * 128 : (l + 1) * 128],
                             rhs=attvT_sb[:], start=True, stop=False)
            nc.tensor.matmul(pc[:], lhsT=wc2T_sb[:, l * 128 : (l + 1) * 128],
                             rhs=gsumT[:], start=False, stop=True)
            nc.scalar.copy(P1[:, l * 128 : (l + 1) * 128], pc[:])

        P1_ap = P1[:]
        P1_lc = P1_ap.rearrange("p (l c) -> p l c", l=H)
        P1_cl = AP(tensor=P1_ap.tensor, offset=P1_ap.offset,
                   ap=[P1_ap.ap[0], [1, 128], [128, H]])

        def squash(v_tile, tag):
            sq = wtile((128, H), f"sq{tag}")
            n2 = wtile((128, 1), f"n2{tag}")
            nc.scalar.activation(sq[:], v_tile[:], AF.Square, accum_out=n2[:])
            st = wtile((128, 1), f"st{tag}")
            nc.scalar.activation(st[:], n2[:], AF.Sqrt, bias=eps_t[:])
            n2p1 = wtile((128, 1), f"n2p1{tag}")
            nc.vector.tensor_scalar_add(n2p1[:], n2[:], 1.0)
            den = wtile((128, 1), f"den{tag}")
            nc.vector.tensor_mul(den[:], n2p1[:], st[:])
            rden = wtile((128, 1), f"rden{tag}")
            nc.vector.reciprocal(rden[:], den[:])
            coef = wtile((128, 1), f"coef{tag}")
            nc.vector.tensor_mul(coef[:], n2[:], rden[:])
            osq = wtile((128, H), f"osq{tag}")
            nc.vector.tensor_scalar_mul(osq[:], v_tile[:], coef[:])
            return osq

        def delta_into(osq, out_tile, accumulate):
            dw = wtile((128, 128 * H), "dw")
            dw3 = dw[:].rearrange("p (c l) -> p c l", l=H)
            nc.vector.tensor_mul(dw3, P1_cl, bc_free(osq[:], [[0, 128], [1, H]]))
            if accumulate:
                dtmp = wtile((128, 128), "dtmp")
                nc.vector.reduce_sum(out=dtmp[:], in_=dw3, axis=X)
                nc.vector.tensor_add(out_tile[:], out_tile[:], dtmp[:])
            else:
                nc.vector.reduce_sum(out=out_tile[:], in_=dw3, axis=X)

        # iter 0: p uniform = 1/128
        o0 = wtile((128, H), "o0")
        nc.vector.reduce_sum(out=o0[:], in_=P1_lc, axis=X)
        o0s = wtile((128, H), "o0s")
        nc.scalar.mul(o0s[:], o0[:], 1.0 / 128.0)
        osq = squash(o0s, "0")
        logits = wtile((128, 128), "logits")
        delta_into(osq, logits, accumulate=False)

        for it in (1, 2):
            p_l = ttile([128, 128])
            nc.tensor.transpose(p_l[:], logits[:], ident[:])
            mxl = wtile((128, 1), "mxl")
            nc.vector.reduce_max(out=mxl[:], in_=p_l[:], axis=X, negate=True)
            exl = wtile((128, 128), "exl")
            nc.scalar.activation(exl[:], p_l[:], AF.Exp, bias=mxl[:])
            sml = wtile((128, 1), "sml")
            nc.vector.reduce_sum(out=sml[:], in_=exl[:], axis=X)
            rsl = wtile((128, 1), "rsl")
            nc.vector.reciprocal(rsl[:], sml[:])
            pco = wtile((128, 128), "pco")
            nc.vector.tensor_scalar_mul(pco[:], exl[:], rsl[:])
            p_p = ttile([128, 128])
            nc.tensor.transpose(p_p[:], pco[:], ident[:])
            pT = wtile((128, 128), "pT")
            nc.scalar.copy(pT[:], p_p[:])
            pw = wtile((128, 128 * H), "pw")
            pw3 = pw[:].rearrange("p (l c) -> p l c", l=H)
            nc.vector.tensor_mul(pw3, P1_lc, bc_free(pT[:], [[0, H], [1, 128]]))
            orr = wtile((128, H), "orr")
            nc.vector.reduce_sum(out=orr[:], in_=pw3, axis=X)
            osq = squash(orr, str(it))
            if it == 1:
                delta_into(osq, logits, accumulate=True)

        # ---- fusion output -----------------------------------------------
        ro = wtile((128, H), "ro")
        nc.scalar.activation(ro[:], osq[:], AF.Relu)
        p_ro = ttile([H, 128])
        nc.tensor.transpose(p_ro[:], ro[:], ident[:])
        roT = wtile((H, 128), "roT")
        nc.scalar.copy(roT[:], p_ro[:])
        p_f = ptile([128, 128], 2)
        nc.tensor.matmul(p_f[:], lhsT=roT[:], rhs=wfusT_sb[:], start=True, stop=True)
        fsum = wtile((128, 128), "fsum")
        nc.vector.tensor_add(fsum[:], p_f[:], bfus_sb[:])
        fout = wtile((128, 128), "fout")
        nc.scalar.activation(fout[:], fsum[:], AF.Tanh)
        nc.sync.dma_start(out=out_d, in_=fout[:])

    nc.compile()
    return nc


# --------------------------------------------------------------------------
# host-side input prep / sharding
# --------------------------------------------------------------------------
def _colpack(vecs):
    """list of (128,) vectors -> (128, len) column array."""
    return np.stack([np.asarray(v, np.float32) for v in vecs], axis=1)


def _prep_in_maps(inputs):
    f32 = lambda k: np.asarray(inputs[k], np.float32)
    x = f32("inputs").reshape(T, F)
    xT_arr = np.ascontiguousarray(x.reshape(T, KT, 128).transpose(2, 1, 0)).reshape(
        128, KT * T
    )

    base = {"xT": xT_arr}
    base["wattT"] = np.ascontiguousarray(f32("W_att").T)
    base["batt"] = f32("b_att").reshape(T, 1)
    for layer, (wl, bl, wr, br, a) in enumerate(
        [("Wl0", "bl0", "Wr0", "br0", "a0"), ("Wl1", "bl1", "Wr1", "br1", "a1")]
    ):
        base[f"wlT{layer}"] = np.ascontiguousarray(f32(wl).T)
        base[f"wrT{layer}"] = np.ascontiguousarray(f32(wr).T)
        base[f"bl{layer}"] = f32(bl).reshape(H, 1)
        base[f"br{layer}"] = f32(br).reshape(H, 1)
        base[f"gb{layer}"] = f32("gb0" if layer == 0 else "gb1").reshape(H, 1)
        base[f"abc{layer}"] = np.tile(0.6 * f32(a).reshape(1, H), (128, 1))
    wc = np.ascontiguousarray(f32("W_caps").transpose(2, 1, 0))  # (2H, H, I)
    base["wc1T"] = np.ascontiguousarray(wc[:H].reshape(H, H * I))
    base["wc2T"] = np.ascontiguousarray(wc[H:].reshape(H, H * I))
    base["wfusT"] = np.ascontiguousarray(f32("W_fus").T)
    base["bfus"] = np.tile(f32("b_fus").reshape(1, I), (I, 1))

    in_maps = []
    for c in range(NCORES):
        m = dict(base)
        fs = slice(c * GPC, (c + 1) * GPC)
        for layer, (wk, bik, bhk) in enumerate(
            [("W_ih0", "b_ih0", "b_hh0"), ("W_ih1", "b_ih1", "b_hh1")]
        ):
            W = f32(wk)
            Wc = np.concatenate([W[g * F : (g + 1) * F][fs] for g in range(3)], axis=0)
            wfull = np.ascontiguousarray(
                Wc.reshape(3 * GPC, KT, 128).transpose(2, 1, 0)
            ).reshape(128, KT * 3 * GPC)
            step = wfull.shape[1] // NCHUNK
            for j in range(NCHUNK):
                m[f"w{layer}_{j}"] = np.ascontiguousarray(
                    wfull[:, j * step : (j + 1) * step]
                )
            bih, bhh = f32(bik), f32(bhk)
            cols = []
            for b in range(2):
                cols.append((bih[0 * F :][fs][b * 128 : (b + 1) * 128]
                             + bhh[0 * F :][fs][b * 128 : (b + 1) * 128]))
            for b in range(2):
                cols.append(-(bih[1 * F :][fs][b * 128 : (b + 1) * 128]
                              + bhh[1 * F :][fs][b * 128 : (b + 1) * 128]))
            for b in range(2):
                cols.append(bih[2 * F :][fs][b * 128 : (b + 1) * 128])
            for b in range(2):
                cols.append(bhh[2 * F :][fs][b * 128 : (b + 1) * 128])
            m[f"bias{layer}"] = _colpack(cols)
        in_maps.append(m)
    return in_maps


def kernel(**inputs):
    if "nc" not in _STATE:
        _STATE["nc"] = _build_module()
    nc = _STATE["nc"]
    in_maps = _prep_in_maps(inputs)

    from concourse.bass_utils import run_bass_kernel_spmd

    trace = bool(int(os.environ.get("KERNEL_TRACE", "0")))
    res = run_bass_kernel_spmd(nc, in_maps, core_ids=list(range(NCORES)), trace=trace)
    _STATE["last_results"] = res
    return np.asarray(res.results[0]["out"], np.float32).reshape(1, I, I)


# revision 11
# speedup vs baseline: 1.5457x; 1.5457x over previous
"""Trainium2 Bass kernel for nn_CapGATattentionGRU (8-core SPMD).

Math notes exploited here:
- The reference GRU scans a length-1 sequence with h0 = 0, so the
  (3F x F) W_hh matmuls reduce to their biases b_hh.  Only W_ih0/W_ih1
  (100 MB total) need to be streamed.
- Tensor-parallel sharding: each core owns 256 output features per gate
  (columns of gi) for both GRU layers; hidden states are AllGathered
  between layers.  Everything after the GRU (attention over T=12, GAT on
  128 nodes, capsule routing) is tiny and runs replicated on all cores.
- All matmul inputs are bf16 (fp32 PSUM accumulation); fp32 matmuls on
  TRN2 run in LOW_HIGH dual-pass mode (~2x columns + no drain overlap),
  measured ~5-10x slower for these shapes.
"""

import os
import numpy as np

I, H, T, F = 128, 16, 12, 2048
NCORES = 8
GPC = F // NCORES          # 256 gate-features per core
KT = F // 128              # 16 k-tiles of the contraction dim
NCHUNK = 4                 # weight DMA chunks per layer

_STATE = {}


# --------------------------------------------------------------------------
# device module
# --------------------------------------------------------------------------
def _build_module():
    from contextlib import ExitStack

    import concourse.bass as bass
    import concourse.tile as tile
    from concourse import bacc, mybir
    from concourse.masks import make_identity

    dt = mybir.dt.float32
    db = mybir.dt.bfloat16
    X = mybir.AxisListType.X
    AF = mybir.ActivationFunctionType
    OP = mybir.AluOpType
    AP = bass.AP

    nc = bacc.Bacc(
        "TRN2",
        target_bir_lowering=False,
        debug=False,
        num_devices=NCORES,
    )

    def din(name, shape, dd=dt):
        return nc.dram_tensor(name, list(shape), dd, kind="ExternalInput").ap()

    w_dram = [
        [din(f"w{layer}_{j}", (128, KT * 3 * GPC // NCHUNK), db) for j in range(NCHUNK)]
        for layer in range(2)
    ]  # each (128, 3072) bf16: k-major [k, j] with j in 0..768
    xT_d = din("xT", (128, KT * T), db)
    bias_d = [din(f"bias{layer}", (128, 8)) for layer in range(2)]
    wattT_d = din("wattT", (T, T), db)
    battbc_d = din("battbc", (128, T))
    gat_d = []
    for layer in range(2):
        gat_d.append(
            dict(
                wlT=din(f"wlT{layer}", (H, H), db),
                wrT=din(f"wrT{layer}", (H, H), db),
                bl=din(f"bl{layer}", (H, 1)),
                br=din(f"br{layer}", (H, 1)),
                gb=din(f"gb{layer}", (H, 1)),
                abc=din(f"abc{layer}", (128, H)),
            )
        )
    wc1T_d = din("wc1T", (H, 2048), db)
    wc2T_d = din("wc2T", (H, 2048), db)
    wfusT_d = din("wfusT", (H, 128), db)
    bfus_d = din("bfus", (128, 128))
    out_d = nc.dram_tensor("out", [128, 128], dt, kind="ExternalOutput").ap()

    with ExitStack() as ctx:
        tc = ctx.enter_context(tile.TileContext(nc))
        const = ctx.enter_context(tc.tile_pool(name="const", bufs=1))
        work = ctx.enter_context(tc.tile_pool(name="work", bufs=1))
        psum = ctx.enter_context(tc.tile_pool(name="psum", bufs=1, space="PSUM"))
        dram = ctx.enter_context(tc.tile_pool(name="dram", bufs=1, space="DRAM"))

        def wtile(shape, tag, dd=dt):
            return work.tile(list(shape), dd, tag=tag, name=tag)

        def ptile(shape, i, dd=dt):
            return psum.tile(list(shape), dd, tag=f"P{i}", name=f"P{i}")

        def ttile(shape, dd=dt):
            return psum.tile(list(shape), dd, tag="tr", name="tr", bufs=2)

        def bc_free(ap, dims):
            """Manual AP: keep partition dim, replace free dims with [step,count]s."""
            return AP(tensor=ap.tensor, offset=ap.offset, ap=[ap.ap[0]] + list(dims))

        # ---- big weight DMAs first (HWDGE, streams in order) -------------
        w_sb = []
        for layer in range(2):
            tiles = []
            for j in range(NCHUNK):
                t = const.tile([128, KT * 3 * GPC // NCHUNK], db,
                               tag=f"w{layer}_{j}", name=f"w{layer}_{j}")
                nc.sync.dma_start(out=t[:], in_=w_dram[layer][j])
                tiles.append(t)
            w_sb.append(tiles)

        # ---- small constants (SWDGE queues, overlap the weight stream) ---
        def load(ap_dram, tag):
            t = const.tile(list(ap_dram.shape), ap_dram.dtype, tag=tag, name=tag)
            nc.gpsimd.dma_start(out=t[:], in_=ap_dram)
            return t

        xT_sb = load(xT_d, "xT")
        bias_sb = [load(bias_d[0], "bias0"), load(bias_d[1], "bias1")]
        wattT_sb = load(wattT_d, "wattT")
        battbc_sb = load(battbc_d, "battbc")
        gat_sb = []
        for layer in range(2):
            gat_sb.append({k: load(v, f"gat{layer}_{k}") for k, v in gat_d[layer].items()})
        wc1T_sb = load(wc1T_d, "wc1T")
        wc2T_sb = load(wc2T_d, "wc2T")
        wfusT_sb = load(wfusT_d, "wfusT")
        bfus_sb = load(bfus_d, "bfus")

        ident = const.tile([128, 128], dt, tag="ident", name="ident")
        make_identity(nc, ident[:])
        identb = const.tile([128, 128], db, tag="identb", name="identb")
        make_identity(nc, identb[:])
        ones1 = const.tile([1, 128], db, tag="ones1", name="ones1")
        nc.vector.memset(ones1[:], 1.0)
        eps_t = const.tile([128, 1], dt, tag="eps_t", name="eps_t")
        nc.vector.memset(eps_t[:], 1e-8)

        # ---- GRU layers --------------------------------------------------
        h1T_sb = wtile((128, KT * T), "h1T", db)
        embT_bf = wtile((128, KT * T), "embT", db)
        # layer 0 gathers the (f, t) layout only; layer 1 additionally
        # gathers pre-transposed (t, f) blocks so emb lands in both layouts
        # with no post-gather transposes.
        d_slice = [
            dram.tile([2 * 128, T], db, tag="dsl0", name="dsl0"),
            dram.tile([2 * 128 + 2 * 128, T], db, tag="dsl1", name="dsl1"),
        ]
        d_full = [
            dram.tile([F, T], db, tag="dfull0", name="dfull0"),
            dram.tile([2 * F, T], db, tag="dfull1", name="dfull1"),
        ]

        for layer in range(2):
            rhs3 = (xT_sb if layer == 0 else h1T_sb)[:].rearrange(
                "p (k t) -> p k t", k=KT
            )
            ps = [ptile([128, T], g * 2 + b) for g in range(3) for b in range(2)]
            for k in range(KT):
                ch, kk = k // 4, k % 4
                wv = w_sb[layer][ch][:].rearrange("p (k2 j) -> p k2 j", k2=4)
                for g in range(3):
                    for b in range(2):
                        nc.tensor.matmul(
                            ps[g * 2 + b][:],
                            lhsT=wv[:, kk, g * GPC + b * 128 : g * GPC + (b + 1) * 128],
                            rhs=rhs3[:, k, :],
                            start=(k == 0),
                            stop=(k == KT - 1),
                        )
            bl_sb = bias_sb[layer]
            for b in range(2):
                r_t = wtile((128, T), f"r{b}")
                nc.scalar.activation(r_t[:], ps[0 * 2 + b][:], AF.Sigmoid,
                                     bias=bl_sb[:, 0 + b : 1 + b])
                zc_t = wtile((128, T), f"zc{b}")
                nc.scalar.activation(zc_t[:], ps[1 * 2 + b][:], AF.Sigmoid,
                                     bias=bl_sb[:, 2 + b : 3 + b], scale=-1.0)
                t_t = wtile((128, T), f"t{b}")
                nc.vector.scalar_tensor_tensor(
                    out=t_t[:], in0=r_t[:], scalar=bl_sb[:, 6 + b : 7 + b],
                    in1=ps[2 * 2 + b][:], op0=OP.mult, op1=OP.add,
                )
                n_t = wtile((128, T), f"n{b}")
                nc.scalar.activation(n_t[:], t_t[:], AF.Tanh,
                                     bias=bl_sb[:, 4 + b : 5 + b])
                h_t = wtile((128, T), f"h{b}", db)
                if layer == 0:
                    nc.vector.tensor_mul(h_t[:], zc_t[:], n_t[:])
                else:
                    hf_t = wtile((128, T), f"hf{b}")
                    nc.vector.tensor_mul(hf_t[:], zc_t[:], n_t[:])
                    nc.scalar.activation(h_t[:], hf_t[:], AF.Relu)  # emb = relu
                nc.sync.dma_start(
                    out=d_slice[layer][b * 128 : (b + 1) * 128, :], in_=h_t[:]
                )
                if layer == 1:
                    # also ship the (t, f) layout: transpose the relu'd block
                    trh = ttile([T, 128], db)
                    nc.tensor.transpose(trh[:], h_t[:], identb[:])
                    hn_t = wtile((T, 128), f"hn{b}", db)
                    nc.scalar.copy(hn_t[:], trh[:])
                    nat = d_slice[1][256 + b * 128 : 256 + (b + 1) * 128, :]
                    nat = nat.rearrange("a b -> (a b)").rearrange(
                        "(t j) -> t j", t=T
                    )  # (12, 128) region, t-major
                    nc.sync.dma_start(out=nat, in_=hn_t[:])
            nc.gpsimd.collective_compute(
                "AllGather",
                OP.bypass,
                replica_groups=[list(range(NCORES))],
                ins=[d_slice[layer][:].opt()],
                outs=[d_full[layer][:].opt()],
            )
            if layer == 0:
                nc.sync.dma_start(
                    out=h1T_sb[:].rearrange("p (k t) -> p k t", k=KT),
                    in_=d_full[0][:].rearrange("(k p) t -> p k t", p=128),
                )

        # d_full[1] is (8 cores) x [2*128 rows of (f,t) slice | 2*128 rows of
        # flat (t,256) nat slice].  Element layouts (flat offsets, bf16):
        #   fT half:  core c, j in [0,256), t:   c*6144 + j*12 + t
        #   nat half: core c, t, j in [0,256):   c*6144 + 3072 + t*256 + j
        full1 = d_full[1][:].rearrange("a b -> (a b)")
        embT3v = embT_bf[:].rearrange("p (c u t) -> p c u t", c=8, u=2)
        for u in range(2):
            embT_in = AP(tensor=full1.tensor, offset=full1.offset + u * 1536,
                         ap=[[12, 128], [6144, 8], [1, 12]])
            nc.sync.dma_start(out=embT3v[:, :, u, :], in_=embT_in)
        emb_nat = wtile((T, F), "emb_nat", db)
        embnat_in = AP(tensor=full1.tensor, offset=full1.offset + 3072,
                       ap=[[256, 12], [6144, 8], [1, 256]])
        nc.sync.dma_start(
            out=emb_nat[:].rearrange("t (c j) -> t c j", c=8),
            in_=embnat_in,
        )

        # ---- attention over T --------------------------------------------
        # aw[f, t'] = sum_t emb[t, f] W_att[t', t] computed directly in
        # (f-part, t'-free) orientation: lhsT = emb_nat slice, rhs = W_att^T.
        aw_sb = wtile((128, KT * T), "aw")
        aw3 = aw_sb[:].rearrange("p (k t) -> p k t", k=KT)
        for k in range(KT):
            pa = ttile([128, T])
            nc.tensor.matmul(pa[:], lhsT=emb_nat[:, k * 128 : (k + 1) * 128],
                             rhs=wattT_sb[:], start=True, stop=True)
            nc.scalar.copy(aw3[:, k, :], pa[:])
        battbc_ap = battbc_sb[:]
        nc.vector.tensor_tensor(
            aw3, aw3, bc_free(battbc_ap, [[0, KT], [1, T]]), OP.add
        )

        mx = wtile((128, KT), "mx")
        nc.vector.reduce_max(out=mx[:], in_=aw3, axis=X)
        exs = wtile((128, KT * T), "exs")
        ex3 = exs[:].rearrange("p (k t) -> p k t", k=KT)
        nc.vector.tensor_tensor(ex3, aw3, bc_free(mx[:], [[1, KT], [0, T]]),
                                OP.subtract)
        nc.scalar.activation(exs[:], exs[:], AF.Exp)
        sm = wtile((128, KT), "sm")
        nc.vector.reduce_sum(out=sm[:], in_=ex3, axis=X)
        rs = wtile((128, KT), "rs")
        nc.vector.reciprocal(rs[:], sm[:])
        embT_f = wtile((128, KT * T), "embT_f")
        nc.scalar.copy(embT_f[:], embT_bf[:])
        pe = wtile((128, KT * T), "pe")
        nc.vector.tensor_mul(pe[:], exs[:], embT_f[:])
        num = wtile((128, KT), "num")
        nc.vector.reduce_sum(out=num[:], in_=pe[:].rearrange("p (k t) -> p k t", k=KT),
                             axis=X)
        attn = wtile((128, KT), "attn")
        nc.vector.tensor_mul(attn[:], num[:], rs[:])
        att_sb = wtile((128, KT), "att_sb", db)
        nc.scalar.activation(att_sb[:], attn[:], AF.Tanh)

        # (p, k) layout -> att_vec (i, h) via DRAM roundtrip
        trv = ttile([KT, 128], db)
        nc.tensor.transpose(trv[:], att_sb[:], identb[:])
        attT_sb = wtile((KT, 128), "attT", db)
        nc.scalar.copy(attT_sb[:], trv[:])
        d_att = dram.tile([KT, 128], db, tag="datt", name="datt")
        nc.sync.dma_start(out=d_att[:], in_=attT_sb[:])
        att_vec = wtile((128, H), "att_vec", db)
        nc.sync.dma_start(
            out=att_vec[:],
            in_=d_att[:].rearrange("a b -> (a b)").rearrange("(i h) -> i h", h=H),
        )
        trv2 = ttile([H, 128], db)
        nc.tensor.transpose(trv2[:], att_vec[:], identb[:])
        attvT_sb = wtile((H, 128), "attvT", db)
        nc.scalar.copy(attvT_sb[:], trv2[:])

        # ---- GATv2 x2 ----------------------------------------------------
        def gat_layer(XT_tile, prm, tag):
            p_xl = ptile([H, 128], 0)
            nc.tensor.matmul(p_xl[:], lhsT=prm["wlT"][:], rhs=XT_tile[:],
                             start=True, stop=True)
            xlT = wtile((H, 128), f"xlT{tag}", db)
            nc.scalar.activation(xlT[:], p_xl[:], AF.Identity, bias=prm["bl"][:])
            p_xr = ptile([H, 128], 1)
            nc.tensor.matmul(p_xr[:], lhsT=prm["wrT"][:], rhs=XT_tile[:],
                             start=True, stop=True)
            xrT = wtile((H, 128), f"xrT{tag}", db)
            nc.scalar.activation(xrT[:], p_xr[:], AF.Identity, bias=prm["br"][:])

            p_t = ttile([128, H], db)
            nc.tensor.transpose(p_t[:], xlT[:], identb[0:H, 0:H])
            xl_nat = wtile((128, H), f"xln{tag}", db)
            nc.scalar.copy(xl_nat[:], p_t[:])

            xlf = wtile((1, 128 * H), "xlf", db)
            xlf_ap = xlf[:]
            nc.sync.dma_start(
                out=AP(tensor=xlf_ap.tensor, offset=xlf_ap.offset,
                       ap=[xlf_ap.ap[0], [H, 128], [1, H]]),
                in_=xl_nat[:],
            )
            e_sb = wtile((128, 128 * H), "e_sb")
            id_ap = identb[0:H, 0:H]
            id_rep = AP(tensor=id_ap.tensor, offset=id_ap.offset,
                        ap=[id_ap.ap[0], [0, 512 // H], id_ap.ap[1]])
            for j in range(4):
                p_e = ptile([128, 512], 2 + j)
                nc.tensor.matmul(p_e[:], lhsT=ones1[:],
                                 rhs=xlf[0:1, j * 512 : (j + 1) * 512],
                                 start=True, stop=False)
                nc.tensor.matmul(p_e[:], lhsT=xrT[:], rhs=id_rep,
                                 start=False, stop=True)
                # lrelu(v, 0.2) = 0.6*(v + (2/3)*|v|); the 0.6 is folded into
                # the host-side scaling of `a` (abc input carries 0.6*a).
                ab_t = wtile((128, 512), f"ab{j}")
                nc.scalar.activation(ab_t[:], p_e[:], AF.Abs)
                nc.vector.scalar_tensor_tensor(
                    out=e_sb[:, j * 512 : (j + 1) * 512], in0=ab_t[:],
                    scalar=2.0 / 3.0, in1=p_e[:], op0=OP.mult, op1=OP.add,
                )
            ew = wtile((128, 128 * H), "ew")
            abc_ap = prm["abc"][:]
            nc.vector.tensor_mul(
                ew[:].rearrange("p (s h) -> p s h", h=H),
                e_sb[:].rearrange("p (s h) -> p s h", h=H),
                bc_free(abc_ap, [[0, 128], [1, H]]),
            )
            spre = wtile((128, 128), "spre")
            nc.vector.reduce_sum(out=spre[:],
                                 in_=ew[:].rearrange("p (s h) -> p s h", h=H), axis=X)
            mx2 = wtile((128, 1), "mx2")
            nc.vector.reduce_max(out=mx2[:], in_=spre[:], axis=X, negate=True)
            ex2 = wtile((128, 128), "ex2")
            nc.scalar.activation(ex2[:], spre[:], AF.Exp, bias=mx2[:])
            sm2 = wtile((128, 1), "sm2")
            nc.vector.reduce_sum(out=sm2[:], in_=ex2[:], axis=X)
            rs2 = wtile((128, 1), "rs2")
            nc.vector.reciprocal(rs2[:], sm2[:])
            alph = wtile((128, 128), "alph", db)
            nc.vector.tensor_scalar_mul(alph[:], ex2[:], rs2[:])
            p_at = ttile([128, 128], db)
            nc.tensor.transpose(p_at[:], alph[:], identb[:])
            alphT = wtile((128, 128), "alphT", db)
            nc.scalar.copy(alphT[:], p_at[:])
            p_g = ptile([H, 128], 0)
            nc.tensor.matmul(p_g[:], lhsT=xl_nat[:], rhs=alphT[:],
                             start=True, stop=True)
            gT = wtile((H, 128), f"gT{tag}", db)
            nc.scalar.activation(gT[:], p_g[:], AF.Relu, bias=prm["gb"][:])
            return gT

        g0T = gat_layer(attvT_sb, gat_sb[0], "0")
        g1T = gat_layer(g0T, gat_sb[1], "1")
        gsumT = wtile((H, 128), "gsumT", db)
        nc.vector.tensor_add(gsumT[:], g0T[:], g1T[:])

        # ---- capsule priors + routing ------------------------------------
        P1 = wtile((128, H * 128), "P1")  # [o, (l, c)] fp32
        for l in range(H):
            pc = ptile([128, 128], l % 2)
            nc.tensor.matmul(pc[:], lhsT=wc1T_sb[:, l * 128 : (l + 1) * 128],
                             rhs=attvT_sb[:], start=True, stop=False)
            nc.tensor.matmul(pc[:], lhsT=wc2T_sb[:, l * 128 : (l + 1) * 128],
                             rhs=gsumT[:], start=False, stop=True)
            nc.scalar.copy(P1[:, l * 128 : (l + 1) * 128], pc[:])

        P1_ap = P1[:]
        P1_lc = P1_ap.rearrange("p (l c) -> p l c", l=H)
        P1_cl = AP(tensor=P1_ap.tensor, offset=P1_ap.offset,
                   ap=[P1_ap.ap[0], [1, 128], [128, H]])

        def squash(v_tile, tag):
            sq = wtile((128, H), f"sq{tag}")
            n2 = wtile((128, 1), f"n2{tag}")
            nc.scalar.activation(sq[:], v_tile[:], AF.Square, accum_out=n2[:])
            st = wtile((128, 1), f"st{tag}")
            nc.scalar.activation(st[:], n2[:], AF.Sqrt, bias=eps_t[:])
            n2p1 = wtile((128, 1), f"n2p1{tag}")
            nc.vector.tensor_scalar_add(n2p1[:], n2[:], 1.0)
            den = wtile((128, 1), f"den{tag}")
            nc.vector.tensor_mul(den[:], n2p1[:], st[:])
            rden = wtile((128, 1), f"rden{tag}")
            nc.vector.reciprocal(rden[:], den[:])
            coef = wtile((128, 1), f"coef{tag}")
            nc.vector.tensor_mul(coef[:], n2[:], rden[:])
            osq = wtile((128, H), f"osq{tag}")
            nc.vector.tensor_scalar_mul(osq[:], v_tile[:], coef[:])
            return osq

        def delta_into(osq, out_tile, accumulate):
            dw = wtile((128, 128 * H), "dw")
            dw3 = dw[:].rearrange("p (c l) -> p c l", l=H)
            nc.vector.tensor_mul(dw3, P1_cl, bc_free(osq[:], [[0, 128], [1, H]]))
            if accumulate:
                dtmp = wtile((128, 128), "dtmp")
                nc.vector.reduce_sum(out=dtmp[:], in_=dw3, axis=X)
                nc.vector.tensor_add(out_tile[:], out_tile[:], dtmp[:])
            else:
                nc.vector.reduce_sum(out=out_tile[:], in_=dw3, axis=X)

        # iter 0: p uniform = 1/128
        o0 = wtile((128, H), "o0")
        nc.vector.reduce_sum(out=o0[:], in_=P1_lc, axis=X)
        o0s = wtile((128, H), "o0s")
        nc.scalar.mul(o0s[:], o0[:], 1.0 / 128.0)
        osq = squash(o0s, "0")
        logits = wtile((128, 128), "logits")
        delta_into(osq, logits, accumulate=False)

        for it in (1, 2):
            p_l = ttile([128, 128])
            nc.tensor.transpose(p_l[:], logits[:], ident[:])
            mxl = wtile((128, 1), "mxl")
            nc.vector.reduce_max(out=mxl[:], in_=p_l[:], axis=X, negate=True)
            exl = wtile((128, 128), "exl")
            nc.scalar.activation(exl[:], p_l[:], AF.Exp, bias=mxl[:])
            sml = wtile((128, 1), "sml")
            nc.vector.reduce_sum(out=sml[:], in_=exl[:], axis=X)
            rsl = wtile((128, 1), "rsl")
            nc.vector.reciprocal(rsl[:], sml[:])
            pco = wtile((128, 128), "pco")
            nc.vector.tensor_scalar_mul(pco[:], exl[:], rsl[:])
            p_p = ttile([128, 128])
            nc.tensor.transpose(p_p[:], pco[:], ident[:])
            pT = wtile((128, 128), "pT")
            nc.scalar.copy(pT[:], p_p[:])
            pw = wtile((128, 128 * H), "pw")
            pw3 = pw[:].rearrange("p (l c) -> p l c", l=H)
            nc.vector.tensor_mul(pw3, P1_lc, bc_free(pT[:], [[0, H], [1, 128]]))
            orr = wtile((128, H), "orr")
            nc.vector.reduce_sum(out=orr[:], in_=pw3, axis=X)
            osq = squash(orr, str(it))
            if it == 1:
                delta_into(osq, logits, accumulate=True)

        # ---- fusion output -----------------------------------------------
        ro = wtile((128, H), "ro", db)
        nc.scalar.activation(ro[:], osq[:], AF.Relu)
        p_ro = ttile([H, 128], db)
        nc.tensor.transpose(p_ro[:], ro[:], identb[:])
        roT = wtile((H, 128), "roT", db)
        nc.scalar.copy(roT[:], p_ro[:])
        p_f = ptile([128, 128], 2)
        nc.tensor.matmul(p_f[:], lhsT=roT[:], rhs=wfusT_sb[:], start=True, stop=True)
        fsum = wtile((128, 128), "fsum")
        nc.vector.tensor_add(fsum[:], p_f[:], bfus_sb[:])
        fout = wtile((128, 128), "fout")
        nc.scalar.activation(fout[:], fsum[:], AF.Tanh)
        nc.sync.dma_start(out=out_d, in_=fout[:])

    nc.compile()
    return nc


# --------------------------------------------------------------------------
# host-side input prep / sharding
# --------------------------------------------------------------------------
def _colpack(vecs):
    """list of (128,) vectors -> (128, len) column array."""
    return np.stack([np.asarray(v, np.float32) for v in vecs], axis=1)


def _prep_in_maps(inputs):
    import ml_dtypes

    bf16 = ml_dtypes.bfloat16
    f32 = lambda k: np.asarray(inputs[k], np.float32)
    x = f32("inputs").reshape(T, F)
    xT_arr = np.ascontiguousarray(x.reshape(T, KT, 128).transpose(2, 1, 0)).reshape(
        128, KT * T
    )

    base = {"xT": xT_arr.astype(bf16)}
    base["wattT"] = np.ascontiguousarray(f32("W_att").T).astype(bf16)
    base["battbc"] = np.tile(f32("b_att").reshape(1, T), (128, 1))
    for layer, (wl, bl, wr, br, a, gb) in enumerate(
        [("Wl0", "bl0", "Wr0", "br0", "a0", "gb0"),
         ("Wl1", "bl1", "Wr1", "br1", "a1", "gb1")]
    ):
        base[f"wlT{layer}"] = np.ascontiguousarray(f32(wl).T).astype(bf16)
        base[f"wrT{layer}"] = np.ascontiguousarray(f32(wr).T).astype(bf16)
        base[f"bl{layer}"] = f32(bl).reshape(H, 1)
        base[f"br{layer}"] = f32(br).reshape(H, 1)
        base[f"gb{layer}"] = f32(gb).reshape(H, 1)
        base[f"abc{layer}"] = np.tile(0.6 * f32(a).reshape(1, H), (128, 1))
    wc = np.ascontiguousarray(f32("W_caps").transpose(2, 1, 0))  # (2H, H, I)
    base["wc1T"] = np.ascontiguousarray(wc[:H].reshape(H, H * I)).astype(bf16)
    base["wc2T"] = np.ascontiguousarray(wc[H:].reshape(H, H * I)).astype(bf16)
    base["wfusT"] = np.ascontiguousarray(f32("W_fus").T).astype(bf16)
    base["bfus"] = np.tile(f32("b_fus").reshape(1, I), (I, 1))

    in_maps = []
    for c in range(NCORES):
        m = dict(base)
        fs = slice(c * GPC, (c + 1) * GPC)
        for layer, (wk, bik, bhk) in enumerate(
            [("W_ih0", "b_ih0", "b_hh0"), ("W_ih1", "b_ih1", "b_hh1")]
        ):
            W = f32(wk)
            Wc = np.concatenate([W[g * F : (g + 1) * F][fs] for g in range(3)], axis=0)
            wfull = np.ascontiguousarray(
                Wc.reshape(3 * GPC, KT, 128).transpose(2, 1, 0)
            ).reshape(128, KT * 3 * GPC).astype(bf16)
            step = wfull.shape[1] // NCHUNK
            for j in range(NCHUNK):
                m[f"w{layer}_{j}"] = np.ascontiguousarray(
                    wfull[:, j * step : (j + 1) * step]
                )
            bih, bhh = f32(bik), f32(bhk)
            cols = []
            for b in range(2):
                cols.append((bih[0 * F :][fs][b * 128 : (b + 1) * 128]
                             + bhh[0 * F :][fs][b * 128 : (b + 1) * 128]))
            for b in range(2):
                cols.append(-(bih[1 * F :][fs][b * 128 : (b + 1) * 128]
                              + bhh[1 * F :][fs][b * 128 : (b + 1) * 128]))
            for b in range(2):
                cols.append(bih[2 * F :][fs][b * 128 : (b + 1) * 128])
            for b in range(2):
                cols.append(bhh[2 * F :][fs][b * 128 : (b + 1) * 128])
            m[f"bias{layer}"] = _colpack(cols)
        in_maps.append(m)
    return in_maps


def kernel(**inputs):
    if "nc" not in _STATE:
        _STATE["nc"] = _build_module()
    nc = _STATE["nc"]
    in_maps = _prep_in_maps(inputs)

    from concourse.bass_utils import run_bass_kernel_spmd

    trace = bool(int(os.environ.get("KERNEL_TRACE", "0")))
    res = run_bass_kernel_spmd(nc, in_maps, core_ids=list(range(NCORES)), trace=trace)
    _STATE["last_results"] = res
    return np.asarray(res.results[0]["out"], np.float32).reshape(1, I, I)
